# revision 30
# baseline (speedup 1.0000x reference)
"""Multi-head attention (B=2, S=2048, D=1024, H=16, causal) on 8 TRN2 NeuronCores.

Sharding: 8 cores = 2 batches x 4 head-groups (4 heads each).  Each core
computes the QKV projections for its head slice, causal attention for its 4
heads, and the partial output projection (input-dim slice of Wo).  The
all-reduce over head-groups happens at gather time on the host (sum of 4
partials per batch), which is the "all-reduce after the output projection"
of a tensor-parallel split.

Everything on device works in token-transposed layout ([feature, token]) so
no on-device transposes are needed:
  scores^T[kv, q] = K_projT_tile^T @ Q_projT   (K = dh = 64)
  P = exp(scores^T)  (no max subtraction needed: scores ~ N(0,1), |s| < ~7)
  out^T[dh(+1), q] = [V | ones]^T @ P          (ones column -> softmax denom)
  partial^T[dmodel, tok] = WoT_slice^T @ attn_out^T

Perf structure (v2 schedule):
  - inputs are host-swizzled so DMA arrives in compute order: q/k in
    512-token q-tile slices [128, 8kt, 512], v in 128-token kv-tile slices
    [128, 8kt, 128]; the first projection group starts ~4us earlier.
  - warmup matmuls at t=0 keep the PE HAM activity monitor busy so the
    clock gate opens (1.2 -> 2.4 GHz) before real work lands.
  - all work (QKV projections, both head-pair attention chains, O
    projection) is interleaved at kv-tile granularity in one long tensor
    queue, so the PE never idles >3.4us and exp (scalar engine) overlaps
    matmul throughout.
  - exp for the two packed heads is issued as ONE activation over a
    2-bank PSUM tile ([128, 2, 512]) halving ACT instruction overhead.
  - softmax denominator: reciprocal reads the PSUM ones-row directly and
    gpsimd broadcasts it; no SBUF->SBUF DMA round-trip.
  - output partials are cast to bf16 before the store DMA (half traffic);
    host sums in f32.
"""

import math
import os

import numpy as np
import ml_dtypes

_BF16 = ml_dtypes.bfloat16

B, S, D = 2, 2048, 1024
H, DH = 16, 64
NCORES = 8
GRP = 4  # heads per core
KT = D // 128  # 8 k-tiles over d_model
NQ = 512  # q tile width (free dim of score tiles)
QTILES = S // NQ  # 4
KVTILES = S // 128  # 16

last_results = None

_programs = {}


def _build_program(causal: bool):
    WARM = int(os.environ.get("KOPT_WARM", "64"))

    import concourse.bass as bass
    import concourse.mybir as mybir
    import concourse.tile as tile
    from concourse import bacc

    f32 = mybir.dt.float32
    bf16 = mybir.dt.bfloat16
    Exp = mybir.ActivationFunctionType.Exp
    Copy = mybir.ActivationFunctionType.Copy

    nc = bacc.Bacc(
        "TRN2",
        target_bir_lowering=False,
        debug=False,
        enable_asserts=False,
        num_devices=NCORES,
    )

    # host-swizzled inputs: qTs[p, n, kt, c] = q^T[128*kt+p, 512*n+c]
    qTs = nc.dram_tensor("qTs", [128, QTILES, KT, NQ], bf16, kind="ExternalInput").ap()
    kTs = nc.dram_tensor("kTs", [128, QTILES, KT, NQ], bf16, kind="ExternalInput").ap()
    # vTs[p, mt, kt, c] = v^T[128*kt+p, 128*mt+c]
    vTs = nc.dram_tensor("vTs", [128, KVTILES, KT, 128], bf16, kind="ExternalInput").ap()
    # weights host-swizzled to [128, kt, 256] so one full-rate DMA loads each
    wqs_d = nc.dram_tensor("wqs", [128, KT, 256], bf16, kind="ExternalInput").ap()
    wks_d = nc.dram_tensor("wks", [128, KT, 256], bf16, kind="ExternalInput").ap()
    wvs_d = nc.dram_tensor("wvs", [128, KT, 256], bf16, kind="ExternalInput").ap()
    woT = nc.dram_tensor("woT", [256, D], bf16, kind="ExternalInput").ap()
    if not causal:
        maskT = nc.dram_tensor("maskT", [S, S], bf16, kind="ExternalInput").ap()
    out = nc.dram_tensor("out", [D, S], bf16, kind="ExternalOutput").ap()

    with tile.TileContext(nc) as tc:
        with (
            tc.tile_pool(name="persist", bufs=1) as sb,
            tc.tile_pool(name="stream", bufs=3) as stream,
            tc.tile_pool(name="psum", bufs=1, space="PSUM") as psum,
            tc.tile_pool(name="p_sb", bufs=4) as pbuf,
            tc.tile_pool(name="r_sb", bufs=4) as rpool,
            tc.tile_pool(name="m_sb", bufs=4) as mpool,
            tc.tile_pool(name="o_sb", bufs=4) as opool,
        ):
            # ---- persistent SBUF tensors ----
            wq_sb = sb.tile([128, KT, 256], bf16)
            wk_sb = sb.tile([128, KT, 256], bf16)
            wv_sb = sb.tile([128, KT, 256], bf16)
            wo2 = sb.tile([128, 2, D], bf16)  # head h at rows 64*(h%2), chunk h//2
            wo2b = sb.tile([64, D], bf16)  # chunk-1 odd head again at rows 0..63
            qproj = sb.tile([128, 2, S], bf16)
            kproj = sb.tile([128, 2, S], bf16)
            vproj = sb.tile([128, KVTILES, GRP, 66], bf16)
            attn2 = sb.tile([128, 2, S], bf16)  # head h at rows 64*(h%2), chunk h//2

            # ---- warmup: keep the PE HAM window busy from t=0 so the
            # clock is at 2.4 GHz when the first real matmul lands ----
            if WARM:
                wz = sb.tile([128, 128], bf16)
                nc.gpsimd.memset(wz[:], 0.0)
                warm_ps = psum.tile([65, NQ], f32, tag="av", bufs=2, name="warm")
                for _ in range(WARM):
                    nc.tensor.matmul(
                        warm_ps[:, 0:128], wz[:, 0:65], wz[:], start=True, stop=True
                    )

            if causal:
                # single 128x128 causal block: keep where q_local >= kv_local
                mask128 = sb.tile([128, 128], bf16)
                nc.gpsimd.memset(mask128[:], 1.0)
                nc.gpsimd.affine_select(
                    out=mask128[:],
                    in_=mask128[:],
                    compare_op=mybir.AluOpType.is_ge,
                    fill=0.0,
                    base=0,
                    pattern=[[1, 128]],
                    channel_multiplier=-1,
                )

            # ones columns at index 0 and 65 of vproj (V lands in cols 1..64);
            # on the vector engine, which is otherwise idle until ~13us
            nc.vector.memset(vproj[:], 1.0)

            # ---- input DMAs (order = arrival order per queue) ----
            # sync queue:   wq | q n0..n3 | vmt 8..15
            # gpsimd queue: wk | k n0 | wv | vmt 0..3 | k n1 | wo | vmt 4..7 | k n2 | k n3
            qn, kn, vmt = {}, {}, {}

            def dma_qn(n):
                t = stream.tile([128, KT, NQ], bf16, tag="qn", bufs=3)
                nc.sync.dma_start(t[:], qTs[:, n, :, :])
                qn[n] = t

            def dma_kn(n, queue=None):
                t = stream.tile([128, KT, NQ], bf16, tag="kn", bufs=3)
                (queue or nc.gpsimd).dma_start(t[:], kTs[:, n, :, :])
                kn[n] = t

            def dma_vmt(mt, queue):
                t = stream.tile([128, KT, 128], bf16, tag="vmt", bufs=16)
                queue.dma_start(t[:], vTs[:, mt, :, :])
                vmt[mt] = t

            # two rings, each payload in first-use order; q0/k0 split in
            # 4-kt halves (4KB/partition lines = full DMA rate) so the
            # first projection group starts as soon as the first half lands
            # fast ring (sync): q side + most v tiles; slow ring
            # (gpsimd SW-DGE): k side + early v.  Each ring in first-use
            # order; q0/k0 split in 4-kt halves (4KB/partition lines) so
            # the first projection group starts when the first half lands.
            q0 = stream.tile([128, KT, NQ], bf16, tag="qn", bufs=3)
            nc.sync.dma_start(wq_sb[:], wqs_d[:])
            nc.sync.dma_start(q0[:, 0:4, :], qTs[:, 0, 0:4, :])
            nc.sync.dma_start(q0[:, 4:8, :], qTs[:, 0, 4:8, :])
            qn[0] = q0
            dma_qn(1)
            for mt in range(4, 8):
                dma_vmt(mt, nc.sync)
            dma_qn(2)
            for mt in range(8, 12):
                dma_vmt(mt, nc.sync)
            dma_qn(3)
            for mt in range(12, 14):
                dma_vmt(mt, nc.sync)

            k0 = stream.tile([128, KT, NQ], bf16, tag="kn", bufs=3)
            nc.gpsimd.dma_start(wk_sb[:], wks_d[:])
            nc.gpsimd.dma_start(k0[:, 0:4, :], kTs[:, 0, 0:4, :])
            nc.gpsimd.dma_start(k0[:, 4:8, :], kTs[:, 0, 4:8, :])
            kn[0] = k0
            nc.gpsimd.dma_start(wv_sb[:], wvs_d[:])
            for mt in range(0, 4):
                dma_vmt(mt, nc.gpsimd)
            dma_kn(1)
            for h in range(GRP):
                base = 64 * (h % 2)
                nc.gpsimd.dma_start(
                    wo2[base : base + 64, h // 2, :], woT[64 * h : 64 * h + 64, :]
                )
            nc.gpsimd.dma_start(wo2b[:], woT[192:256, :])
            dma_kn(2)
            dma_kn(3)
            for mt in range(14, 16):
                dma_vmt(mt, nc.gpsimd)

            # ones columns at index 0 and 65 of vproj (V lands in cols 1..64);
            # on the vector engine, which is otherwise idle until ~13us
            nc.vector.memset(vproj[:], 1.0)

            # ---- input DMAs (order = arrival order per queue) ----
            # sync queue:   wq | q n0..n3 | vmt 8..15
            # gpsimd queue: wk | k n0 | wv | vmt 0..3 | k n1 | wo | vmt 4..7 | k n2 | k n3
            qn, kn, vmt = {}, {}, {}

            def dma_qn(n):
                t = stream.tile([128, KT, NQ], bf16, tag="qn", bufs=3)
                nc.sync.dma_start(t[:], qTs[:, n, :, :])
                qn[n] = t

            def dma_kn(n, queue=None):
                t = stream.tile([128, KT, NQ], bf16, tag="kn", bufs=3)
                (queue or nc.gpsimd).dma_start(t[:], kTs[:, n, :, :])
                kn[n] = t

            def dma_vmt(mt, queue):
                t = stream.tile([128, KT, 128], bf16, tag="vmt", bufs=16)
                queue.dma_start(t[:], vTs[:, mt, :, :])
                vmt[mt] = t

            # two rings, each payload in first-use order; q0/k0 split in
            # 4-kt halves (4KB/partition lines = full DMA rate) so the
            # first projection group starts as soon as the first half lands
            # fast HW ring (sync) carries the q+v side; the slower SW ring
            # (gpsimd) carries only the k side + late payloads.  Each ring
            # strictly in first-use order.
            q0 = stream.tile([128, KT, NQ], bf16, tag="qn", bufs=3)
            nc.sync.dma_start(wq_sb[:], wqs_d[:])
            nc.sync.dma_start(q0[:, 0:4, :], qTs[:, 0, 0:4, :])
            nc.sync.dma_start(q0[:, 4:8, :], qTs[:, 0, 4:8, :])
            qn[0] = q0
            nc.sync.dma_start(wv_sb[:], wvs_d[:])
            for mt in range(0, 4):
                dma_vmt(mt, nc.sync)
            dma_qn(1)
            for mt in range(4, 8):
                dma_vmt(mt, nc.sync)
            dma_qn(2)
            for mt in range(8, 12):
                dma_vmt(mt, nc.sync)
            dma_qn(3)

            k0 = stream.tile([128, KT, NQ], bf16, tag="kn", bufs=3)
            nc.gpsimd.dma_start(wk_sb[:], wks_d[:])
            nc.gpsimd.dma_start(k0[:, 0:4, :], kTs[:, 0, 0:4, :])
            nc.gpsimd.dma_start(k0[:, 4:8, :], kTs[:, 0, 4:8, :])
            kn[0] = k0
            dma_kn(1)
            for h in range(GRP):
                base = 64 * (h % 2)
                nc.gpsimd.dma_start(
                    wo2[base : base + 64, h // 2, :], woT[64 * h : 64 * h + 64, :]
                )
            nc.gpsimd.dma_start(wo2b[:], woT[192:256, :])
            dma_kn(2)
            dma_kn(3)
            for mt in range(12, 16):
                dma_vmt(mt, nc.gpsimd)

            # ---- emit helpers ----
            # build-time safety: a proj tile must be EMITTED before any
            # attention op that reads it, else the read silently sees the
            # memset/stale data (no dependency is created)
            _emitted = set()

            def qkproj(which, m2, n):
                w_sb = wq_sb if which == "q" else wk_sb
                xt = qn[n] if which == "q" else kn[n]
                proj = qproj if which == "q" else kproj
                ps = psum.tile([128, NQ], f32, tag="op", bufs=2)
                for kt in range(KT):
                    nc.tensor.matmul(
                        ps[:],
                        w_sb[:, kt, 128 * m2 : 128 * m2 + 128],
                        xt[:, kt, :],
                        start=(kt == 0),
                        stop=(kt == KT - 1),
                    )
                nc.vector.tensor_copy(proj[:, m2, NQ * n : NQ * n + NQ], ps[:])
                _emitted.add((which, m2, n))

            def vproj_tile(mt):
                ps = psum.tile([128, 256], f32, tag="op", bufs=2)
                for kt in range(KT):
                    nc.tensor.matmul(
                        ps[:],
                        vmt[mt][:, kt, :],
                        wv_sb[:, kt, :],
                        start=(kt == 0),
                        stop=(kt == KT - 1),
                    )
                nc.vector.tensor_copy(
                    vproj[:, mt, :, 1:65],
                    ps[:].rearrange("p (h d) -> p h d", h=GRP),
                )
                _emitted.add(("v", mt))

            # attention chain state per (c2): av accumulators created at t=0
            class Chain:
                pass

            def attn_start(c2, j):
                ch = Chain()
                ch.c2, ch.j = c2, j
                ch.avs = [
                    psum.tile([65, NQ], f32, tag="av", bufs=2, name=f"av{c2}{j}{i}")
                    for i in range(2)
                ]
                ch.ktiles = 4 * j + 4 if causal else KVTILES
                ch.ps = {}
                ch.offs = {}
                return ch

            def attn_score(ch, t):
                c2, j = ch.c2, ch.j
                assert ("k", c2, t // 4) in _emitted, ("score before kproj", c2, ch.j, t)
                assert ("q", c2, j) in _emitted, ("score before qproj", c2, j, t)
                d = t - 4 * j
                off = 128 * d if (causal and d >= 0) else 0
                ch.offs[t] = off
                # merged score psum: [128, 2, NQ] spans two banks
                sp = psum.tile([128, 2, NQ], f32, tag="sc", bufs=2)
                for i in range(2):
                    base = 64 * i
                    nc.tensor.matmul(
                        sp[:, i, off:NQ],
                        kproj[base : base + 64, c2, 128 * t : 128 * t + 128],
                        qproj[base : base + 64, c2, NQ * j + off : NQ * j + NQ],
                        start=True,
                        stop=True,
                    )
                p = pbuf.tile([128, 2, NQ], bf16, tag="p")
                nc.scalar.activation(p[:, :, off:NQ], sp[:, :, off:NQ], Exp)
                if causal:
                    if d >= 0:
                        for i in range(2):
                            nc.vector.tensor_mul(
                                p[:, i, off : off + 128],
                                p[:, i, off : off + 128],
                                mask128[:],
                            )
                else:
                    mt_t = mpool.tile([128, NQ], bf16, tag="mt")
                    nc.sync.dma_start(
                        mt_t[:],
                        maskT[128 * t : 128 * t + 128, NQ * j : NQ * j + NQ],
                    )
                    for i in range(2):
                        nc.vector.tensor_mul(p[:, i, :], p[:, i, :], mt_t[:])
                ch.ps[t] = p

            def attn_av(ch, t):
                c2 = ch.c2
                assert ("v", t) in _emitted, ("AV before vproj", c2, ch.j, t)
                off = ch.offs[t]
                p = ch.ps.pop(t)
                for i in range(2):
                    nc.tensor.matmul(
                        ch.avs[i][:, off:NQ],
                        vproj[:, t, 2 * c2 + i, 1:66],
                        p[:, i, off:NQ],
                        start=(t == 0),
                        stop=(t == ch.ktiles - 1),
                    )

            def attn_norm(ch, last=False):
                # attn2[rows, c2, q] = av[0:64, q] / av[64, q].
                # Non-last rounds: copy av to SBUF first so the PSUM bank
                # frees for the next chain; the recip chain (DMA
                # partition-spread so reciprocal runs 128-wide, on the idle
                # gpsimd ring) then runs off-critical-path.  Last round:
                # read PSUM directly, start the denominator spread first.
                c2, j = ch.c2, ch.j
                avcs = []
                for i in range(2):
                    if last:
                        avcs.append(ch.avs[i])
                    else:
                        avc = mpool.tile([65, NQ], f32, tag="avc", bufs=4)
                        nc.vector.tensor_copy(avc[:], ch.avs[i][:])
                        avcs.append(avc)
                if last:
                    # combined denominator path for the exposed tail: both
                    # heads' denom rows into one [2,NQ] tile -> ONE spread
                    # DMA + reciprocal + return DMA (scalar ring, free at
                    # tail) -> two broadcasts.  The odd head's normalized
                    # output stays in tmpn (no attn2 shift DMA); the last
                    # O-projection reads it via a k=64 matmul vs wo2b.
                    rss = [rpool.tile([1, NQ], f32, tag=f"rs{i}", name=f"rs{i}") for i in range(2)]
                    for i in range(2):
                        nc.vector.tensor_copy(rss[i][:], avcs[i][64:65, :])
                    rq8 = rpool.tile([128, 8], f32, tag="rq8")
                    for i in range(2):
                        nc.scalar.dma_start(rq8[:, 4 * i : 4 * i + 4], rss[i][:])
                    rqr8 = rpool.tile([128, 8], f32, tag="rqr8")
                    nc.vector.reciprocal(rqr8[:], rq8[:])
                    rrs = [rpool.tile([1, NQ], f32, tag=f"rr{i}", name=f"rrx{i}") for i in range(2)]
                    for i in range(2):
                        nc.scalar.dma_start(rrs[i][:], rqr8[:, 4 * i : 4 * i + 4])
                    rbs = []
                    for i in range(2):
                        rb = rpool.tile([64, NQ], f32, tag="rb")
                        nc.gpsimd.partition_broadcast(
                            rb[:], rrs[i][0:1, :], channels=64
                        )
                        rbs.append(rb)
                    nc.vector.tensor_mul(
                        attn2[0:64, c2, NQ * j : NQ * j + NQ],
                        avcs[0][0:64, :],
                        rbs[0][:],
                    )
                    tmpn = sb.tile([64, NQ], bf16)
                    nc.vector.tensor_mul(tmpn[:], avcs[1][0:64, :], rbs[1][:])
                    return tmpn
                rbs = []
                for i in range(2):
                    avc = avcs[i]
                    rq = rpool.tile([128, 4], f32, tag="rq")
                    nc.sync.dma_start(rq[:], avc[64:65, :])
                    rqr = rpool.tile([128, 4], f32, tag="rqr")
                    nc.vector.reciprocal(rqr[:], rq[:])
                    rr = rpool.tile([1, NQ], f32, tag="rr")
                    nc.sync.dma_start(rr[:], rqr[:])
                    rb = rpool.tile([64, NQ], f32, tag="rb")
                    nc.gpsimd.partition_broadcast(rb[:], rr[0:1, :], channels=64)
                    rbs.append(rb)
                for i in range(2):
                    if i == 0:
                        nc.vector.tensor_mul(
                            attn2[0:64, c2, NQ * j : NQ * j + NQ],
                            avcs[i][0:64, :],
                            rbs[i][:],
                        )
                    else:
                        tmpn = rpool.tile([64, NQ], bf16, tag="tmpn")
                        nc.vector.tensor_mul(tmpn[:], avcs[i][0:64, :], rbs[i][:])
                        nc.sync.dma_start(
                            attn2[64:128, c2, NQ * j : NQ * j + NQ], tmpn[:]
                        )
                return None

            _ops = {}

            def oproj_c0(n, m, tag="op"):
                # first half of the Wo contraction (head-pair chunk 0); can
                # run as soon as attn2[:, 0, n] is normalized
                ps = psum.tile(
                    [128, 2, NQ] if tag == "sc" else [128, NQ],
                    f32,
                    tag=tag,
                    bufs=2,
                )
                aps = ps[:, 0, :] if tag == "sc" else ps[:]
                nc.tensor.matmul(
                    aps,
                    wo2[:, 0, 128 * m : 128 * m + 128],
                    attn2[:, 0, NQ * n : NQ * n + NQ],
                    start=True,
                    stop=False,
                )
                _ops[(n, m)] = (ps, aps)

            def oproj_c1(n, m):
                ps, aps = _ops.pop((n, m))
                nc.tensor.matmul(
                    aps,
                    wo2[:, 1, 128 * m : 128 * m + 128],
                    attn2[:, 1, NQ * n : NQ * n + NQ],
                    start=False,
                    stop=True,
                )
                ot = opool.tile([128, NQ], bf16, tag="ot")
                nc.vector.tensor_copy(ot[:], aps)
                nc.sync.dma_start(out[128 * m : 128 * m + 128, NQ * n : NQ * n + NQ], ot[:])

            def oproj_c1_last(n, m, tmpn):
                # chunk-1 contraction split per head: even head from the
                # normalized attn2 lower half, odd head straight from tmpn
                # (both at partitions 0..63) -- no shift DMA needed.  Cast
                # on the scalar engine, free once exp is done.
                ps, aps = _ops.pop((n, m))
                nc.tensor.matmul(
                    aps,
                    wo2[0:64, 1, 128 * m : 128 * m + 128],
                    attn2[0:64, 1, NQ * n : NQ * n + NQ],
                    start=False,
                    stop=False,
                )
                nc.tensor.matmul(
                    aps,
                    wo2b[:, 128 * m : 128 * m + 128],
                    tmpn[:],
                    start=False,
                    stop=True,
                )
                ot = opool.tile([128, NQ], bf16, tag="ot")
                nc.scalar.activation(ot[:], aps, Copy)
                nc.sync.dma_start(out[128 * m : 128 * m + 128, NQ * n : NQ * n + NQ], ot[:])

            def oproj_m(n, m):
                oproj_c0(n, m)
                oproj_c1(n, m)

            # ---- global schedule ----
            # Fillers are emitted BETWEEN attention t-steps so the tensor
            # queue (strict in-order) always has independent matmul work
            # while exp/mask/AV dependencies resolve.  The AV pair for step
            # t is emitted after step t+1's scores + fillers (software
            # pipeline), giving exp/mask a full step of slack.
            def run_round(ch, fillers):
                nt = ch.ktiles
                nf = len(fillers)
                fi = 0
                prev = None
                for t in range(nt):
                    attn_score(ch, t)
                    want = (t + 1) * nf // nt
                    while fi < want:
                        fillers[fi]()
                        fi += 1
                    if prev is not None:
                        attn_av(ch, prev)
                    prev = t
                attn_av(ch, prev)

            F = lambda f, *a: (lambda: f(*a))

            # R0: initial projections (DMA-gated; queue them densely)
            qkproj("q", 0, 0)
            qkproj("q", 1, 0)
            qkproj("k", 0, 0)
            qkproj("k", 1, 0)

            # j = 0 c2=0 | vproj 0..3 paced with AV consumption
            ch0 = attn_start(0, 0)
            run_round(
                ch0,
                [F(vproj_tile, 0), F(vproj_tile, 1), F(vproj_tile, 2), F(vproj_tile, 3)],
            )
            attn_norm(ch0)
            ch1 = attn_start(1, 0)
            run_round(ch1, [F(qkproj, "q", 0, 1), F(qkproj, "q", 1, 1)])
            attn_norm(ch1)

            # j = 1 c2=0 | kproj n1 chunk0 first (scores t>=4 need it),
            # then vproj 4..7 (AVs t=4..7 of THIS round consume them)
            ch0 = attn_start(0, 1)
            run_round(
                ch0,
                [F(qkproj, "k", 0, 1), F(vproj_tile, 4), F(vproj_tile, 5),
                 F(vproj_tile, 6), F(vproj_tile, 7), F(qkproj, "k", 1, 1)],
            )
            attn_norm(ch0)
            # j = 1 c2=1 | qproj n2, oproj(0)
            ch1 = attn_start(1, 1)
            run_round(
                ch1,
                [F(qkproj, "q", 0, 2), F(qkproj, "q", 1, 2)]
                + [F(oproj_m, 0, m) for m in range(8)],
            )
            attn_norm(ch1)

            # j = 2 c2=0 | kproj n2 chunk0, vproj 8..11, qproj n3
            ch0 = attn_start(0, 2)
            run_round(
                ch0,
                [F(qkproj, "k", 0, 2), F(vproj_tile, 8), F(vproj_tile, 9),
                 F(vproj_tile, 10), F(vproj_tile, 11), F(qkproj, "k", 1, 2)],
            )
            attn_norm(ch0)
            # j = 2 c2=1 | oproj(1), qproj n3, kproj n3
            ch1 = attn_start(1, 2)
            run_round(
                ch1,
                [F(qkproj, "q", 0, 3), F(qkproj, "q", 1, 3)]
                + [F(oproj_m, 1, m) for m in range(8)]
                + [F(qkproj, "k", 0, 3), F(qkproj, "k", 1, 3)],
            )
            attn_norm(ch1)

            # j = 3 c2=0 | vproj 12..15 first (AVs t=12..15 here), oproj(2)
            ch0 = attn_start(0, 3)
            run_round(
                ch0,
                [F(vproj_tile, 12), F(vproj_tile, 13), F(vproj_tile, 14), F(vproj_tile, 15)]
                + [F(oproj_m, 2, m) for m in range(8)],
            )
            attn_norm(ch0)
            # j = 3 c2=1 | oproj(3) chunk-0 halves run during the round
            # (their attn2[:,0,n3] input is ready after norm(0,3)); only
            # the chunk-1 halves trail the final norm.  Four groups can be
            # pending: 2 on the op ring + 2 borrowed from the (now idle)
            # sc ring.
            ch1 = attn_start(1, 3)
            run_round(ch1, [F(oproj_c0, 3, 0, "op"), F(oproj_c0, 3, 1, "op")])
            # sc-ring groups only after the round's last score tile: an
            # open accumulation must not be recycled by later sc allocations
            oproj_c0(3, 2, "sc")
            oproj_c0(3, 3, "sc")
            tmpn_last = attn_norm(ch1, last=True)

            # tail: finish oproj(3); chunk-1 via the split (shift-free) path
            for m in range(4):
                oproj_c1_last(3, m, tmpn_last)
            for m in range(4, 8):
                oproj_c0(3, m)
                oproj_c1_last(3, m, tmpn_last)

    nc.compile()
    return nc


def _get_program(causal: bool):
    if causal not in _programs:
        _programs[causal] = _build_program(causal)
    return _programs[causal]


def kernel(query, key, value, mask, Wq, Wk, Wv, Wo):
    global last_results
    from concourse.bass_utils import run_bass_kernel_spmd

    query = np.asarray(query, dtype=np.float32)
    key = np.asarray(key, dtype=np.float32)
    value = np.asarray(value, dtype=np.float32)
    Wq = np.asarray(Wq, dtype=np.float32)
    Wk = np.asarray(Wk, dtype=np.float32)
    Wv = np.asarray(Wv, dtype=np.float32)
    Wo = np.asarray(Wo, dtype=np.float32)
    m2d = np.asarray(mask).reshape(S, S).astype(bool)

    causal = bool(np.array_equal(m2d, np.tril(np.ones((S, S), dtype=bool))))
    nc = _get_program(causal)

    scale = 1.0 / math.sqrt(DH)
    WqT = np.ascontiguousarray((Wq * scale).T).astype(_BF16)
    WkT = np.ascontiguousarray(Wk.T).astype(_BF16)
    WvT = np.ascontiguousarray(Wv.T).astype(_BF16)
    WoT = np.ascontiguousarray(Wo.T).astype(_BF16)

    def swz_w(wT, sl):  # [D, 256] slice -> [128, KT, 256]
        return np.ascontiguousarray(
            np.asarray(wT[:, sl]).reshape(KT, 128, 256).transpose(1, 0, 2)
        )

    def swz_qk(x):  # [S, D] f32 -> [128, QTILES, KT, NQ] bf16
        xT = x.T  # [D, S]
        return np.ascontiguousarray(
            xT.reshape(KT, 128, QTILES, NQ).transpose(1, 2, 0, 3)
        ).astype(_BF16)

    def swz_v(x):  # [S, D] f32 -> [128, KVTILES, KT, 128] bf16
        xT = x.T
        return np.ascontiguousarray(
            xT.reshape(KT, 128, KVTILES, 128).transpose(1, 2, 0, 3)
        ).astype(_BF16)

    qs = [swz_qk(query[b]) for b in range(B)]
    ks = [swz_qk(key[b]) for b in range(B)]
    vs = [swz_v(value[b]) for b in range(B)]
    if not causal:
        maskTb = np.ascontiguousarray(m2d.T).astype(_BF16)

    in_maps = []
    for c in range(NCORES):
        b, g = c // 4, c % 4
        sl = slice(256 * g, 256 * g + 256)
        im = {
            "qTs": qs[b],
            "kTs": ks[b],
            "vTs": vs[b],
            "wqs": swz_w(WqT, sl),
            "wks": swz_w(WkT, sl),
            "wvs": swz_w(WvT, sl),
            "woT": np.ascontiguousarray(WoT[sl, :]),
        }
        if not causal:
            im["maskT"] = maskTb
        in_maps.append(im)

    trace = os.environ.get("KERNEL_PROFILE", "") == "1"
    res = run_bass_kernel_spmd(nc, in_maps, list(range(NCORES)), trace=trace)
    last_results = res

    outp = np.empty((B, S, D), dtype=np.float32)
    for b in range(B):
        acc = res.results[4 * b]["out"].astype(np.float32)
        for g in range(1, 4):
            acc = acc + res.results[4 * b + g]["out"].astype(np.float32)
        outp[b] = acc.T
    return outp
